# revision 2
# baseline (speedup 1.0000x reference)
"""AttnReadout kernel v2 for Trainium2, 8 NeuronCores, data-parallel over batch.

Math (per batch b, head i):
  c[i,e]    = (bu[i,e] + sum_d Wv[i,e,d] * x[b, i, last_nodes[b,i], d]) / 2
  z[t,e]    = sum_d x[b,t,d] * Wu[i,e,d] / 2       (t over O*N = 8192 tokens)
  s[t,e]    = tanh(z[t,e] + c[i,e])                 (sigmoid via tanh identity)
  score[t]  = sum_e (We[i,e]/2) * s[t,e]            (+ const, dropped: softmax
                                                     is shift invariant)
  alpha     = exp(score) / Z
  out[b,i]  = sum_t alpha[t] * x[b,t,:]

Differences vs v1 (120us):
  - wedot: instead of loading each 128-col tanh tile as PE stationary
    (~107ns LDWEIGHTS per 128 tokens, ~55us total), use one-hot-We
    stationary tiles [128,16] (16-col loads) with the tanh tile as a
    256-col moving operand. Scores land chunk-on-partition in PSUM.
    Chunk pairs run concurrently on col groups (0,0)/(0,32).
  - Z: a 129th all-ones column in the xn layout makes the weighted-sum
    matmul also produce Z = sum(alpha); drops ACT accumulator reads.
  - exp bias dropped (shift invariance).
  - tanh reads PSUM in 2048-elem instructions over a 6-bank z ring.
  - scores [48,256] -> bf16 (DVE) -> DMA xbar transposes [16,128]->[128,16]
    -> exp on [128,128] -> alpha in wsum stationary layout.
"""

import numpy as np
import ml_dtypes

import concourse.bacc as bacc
import concourse.tile as tile
from concourse import mybir
from concourse.bass_utils import run_bass_kernel_spmd

BF = ml_dtypes.bfloat16
B, O, N, D = 32, 2, 4096, 128
NCORES = 8
BPC = B // NCORES          # samples per core
T = O * N                  # tokens per sample
NG = 8                     # tanh groups of 1024 per (b, i)
CHUNK = 256                # wedot chunk (one-hot col r = c//2, parity = c%2)
NCH = T // CHUNK           # 32 wedot chunks per (b, i)
NC64 = T // 128            # 64 wsum chunks of 128 tokens
XD = D + 1                 # xn row length: 128 features + ones column


def _build_program():
    nc = bacc.Bacc("TRN2", target_bir_lowering=False)
    dt = mybir.dt
    f32, bf16 = dt.float32, dt.bfloat16

    xt_d = nc.dram_tensor("xt", [BPC, D, T], bf16, kind="ExternalInput")
    xn_d = nc.dram_tensor("xn", [BPC, D, NC64 * XD], bf16, kind="ExternalInput")
    wu_d = nc.dram_tensor("wuT", [D, O, D], bf16, kind="ExternalInput")
    wv_d = nc.dram_tensor("wvT", [D, O, D], bf16, kind="ExternalInput")
    woh_d = nc.dram_tensor("weoh", [D, O, 16, 16], bf16, kind="ExternalInput")
    bu_d = nc.dram_tensor("buR", [D, O * BPC], f32, kind="ExternalInput")
    xl_d = nc.dram_tensor("xlT", [D, O * BPC], bf16, kind="ExternalInput")
    id_d = nc.dram_tensor("id16", [48, 16], bf16, kind="ExternalInput")
    out_d = nc.dram_tensor("out", [BPC, O, D], f32, kind="ExternalOutput")

    Tanh = mybir.ActivationFunctionType.Tanh
    Exp = mybir.ActivationFunctionType.Exp

    with tile.TileContext(nc) as tc:
        from contextlib import ExitStack

        with ExitStack() as ctx:
            singles = ctx.enter_context(tc.tile_pool(name="singles", bufs=1))
            xtp = ctx.enter_context(tc.tile_pool(name="xtp", bufs=3))
            xnp = ctx.enter_context(tc.tile_pool(name="xnp", bufs=3))
            sp = ctx.enter_context(tc.tile_pool(name="sp", bufs=4))
            zp = ctx.enter_context(tc.tile_pool(name="zp", bufs=2, space="PSUM"))
            psp = ctx.enter_context(tc.tile_pool(name="psp", bufs=1, space="PSUM"))
            smalls = ctx.enter_context(tc.tile_pool(name="smalls", bufs=2))
            alphp = ctx.enter_context(tc.tile_pool(name="alphp", bufs=3))

            # PE warm-up source: no DMA dependency, ready immediately
            warm_sb = singles.tile([D, D], bf16)
            nc.gpsimd.memset(warm_sb, 0.001)

            # first group's xt ahead of the weights so the first
            # projection can start early; ch-path weights ride the idle
            # Activation DMA queue instead of the serial Sync queue
            xt0_sb = xtp.tile([D, T], bf16, tag="xt")
            nc.sync.dma_start(out=xt0_sb[:, 0:512], in_=xt_d[0, :, 0:512])
            nc.sync.dma_start(out=xt0_sb[:, 512:1024], in_=xt_d[0, :, 512:1024])

            wu_sb = singles.tile([D, O, D], bf16)
            nc.sync.dma_start(out=wu_sb, in_=wu_d[:])
            wv_sb = singles.tile([D, O, D], bf16)
            nc.scalar.dma_start(out=wv_sb, in_=wv_d[:])
            woh_sb = singles.tile([D, O, 16, 16], bf16)
            nc.sync.dma_start(out=woh_sb, in_=woh_d[:])
            bu_sb = singles.tile([D, O * BPC], f32)
            nc.scalar.dma_start(out=bu_sb, in_=bu_d[:])
            xl_sb = singles.tile([D, O * BPC], bf16)
            nc.scalar.dma_start(out=xl_sb, in_=xl_d[:])
            id_sb = singles.tile([48, 16], bf16)
            nc.sync.dma_start(out=id_sb, in_=id_d[:])

            # PSUM: z double-buffer (2x2 banks via zp pool) + two score
            # banks (one per wedot parity / PE column group) + misc (1 bank)
            # + alpha transpose bank
            scorA = psp.tile([16, CHUNK], f32, tag="scoresA")
            scorB = psp.tile([48, CHUNK], f32, tag="scoresB")
            misc = psp.tile([D, 512], f32, tag="misc")
            alps = psp.tile([D, O, 2, 2, 16], bf16, tag="alps")

            # HAM warmup: dense dummy matmuls fill the otherwise-idle PE
            # during the initial DMA wait so the real stream starts at the
            # un-throttled 2.4 GHz clock (the activity window is ~3.4 us).
            for w in range(16):
                nc.tensor.matmul(
                    misc[:, 0:D],
                    warm_sb,
                    warm_sb,
                    start=(w == 0),
                    stop=(w == 15),
                )

            # per-(sample, head) tanh bias ch[e, j] = (xv + bu)/2, j = i*BPC+b
            # (wv and bu are uploaded pre-halved)
            c_ps = misc[:, 136 : 136 + O * BPC]
            for i in range(O):
                nc.tensor.matmul(
                    c_ps[:, i * BPC : (i + 1) * BPC],
                    wv_sb[:, i, :],
                    xl_sb[:, i * BPC : (i + 1) * BPC],
                    start=True,
                    stop=True,
                )
            ch_sb = singles.tile([D, O * BPC], f32)
            nc.vector.tensor_add(ch_sb, c_ps, bu_sb)

            samples = {}

            def start_sample(b):
                xt_sb = xt0_sb if b == 0 else xtp.tile([D, T], bf16, tag="xt")
                # first sample: fine-grained leading slices so the first
                # projections can start before the bulk of the load lands
                bounds = (
                    [1024, 2048, 4096, 6144, T]
                    if b == 0
                    else [q * (T // 4) for q in range(4)] + [T]
                )
                for lo, hi in zip(bounds[:-1], bounds[1:]):
                    nc.sync.dma_start(out=xt_sb[:, lo:hi], in_=xt_d[b, :, lo:hi])
                xn_sb = xnp.tile([D, NC64, XD], bf16, tag="xn")
                for g in range(2):
                    nc.sync.dma_start(
                        out=xn_sb[:, g * 32 : (g + 1) * 32, :],
                        in_=xn_d[b, :, g * 32 * XD : (g + 1) * 32 * XD].rearrange(
                            "p (c d) -> p c d", c=32
                        ),
                    )
                # alpha tile [128, (parity, q, r, i)]: chunk-major so
                # the wsum stationary [128, 2] slices are contiguous
                aexp_sb = alphp.tile([D, 2, 2, 16, O], bf16, tag="aexp")
                samples[b] = (xt_sb, xn_sb, aexp_sb)

            # wedot: chunk c of (b,i): stationary woh[:, i, r=c//2, :] one-hot
            # (We/2 at col r), moving tanh [128, 256], accumulated into row r
            # of its parity's scores bank. Even/odd chunks pair on PE column
            # groups 0/32 (concurrent execution); separate banks keep the
            # has_written accumulate chains independent.
            def emit_wedot(b, i, g, t_flat):
                for sub in range(4):
                    c = g * 4 + sub
                    r, parity = c // 2, c % 2
                    out_ap = scorA if parity == 0 else scorB[32:48, :]
                    nc.tensor.matmul(
                        out_ap,
                        woh_sb[:, i, r, :],
                        t_flat[:, sub * CHUNK : (sub + 1) * CHUNK],
                        start=(c == parity),
                        stop=(c >= NCH - 2),
                    )
                if g == NG - 1:
                    emit_score_tail(b, i)

            # scores -> bf16 -> PE transposes (into the alps PSUM bank) ->
            # exp from PSUM into the SBUF alpha layout
            def emit_score_tail(b, i):
                aexp_sb = samples[b][2]
                scb = smalls.tile([48, CHUNK], bf16, tag="scb")
                nc.vector.tensor_copy(out=scb[0:16, :], in_=scorA)
                nc.vector.tensor_copy(out=scb[32:48, :], in_=scorB[32:48, :])
                for parity in range(2):
                    for q in range(2):
                        nc.tensor.transpose(
                            out=alps[:, i, parity, q, :],
                            in_=scb[parity * 32 : parity * 32 + 16,
                                    q * 128 : (q + 1) * 128],
                            identity=id_sb[parity * 32 : parity * 32 + 16, :],
                        )
                if i == O - 1:
                    pending_exp.append(b)

            def emit_exp(b):
                aexp_sb = samples[b][2]
                # out AP permutes head-major (transpose layout) into
                # chunk-major (contiguous wsum stationary slices)
                nc.scalar.activation(
                    out=aexp_sb.rearrange("p a b c d -> p d a b c"),
                    in_=alps,
                    func=Exp,
                )
                for blk in range(NC64 // WSB):
                    deferred_wsum.append((b, blk))

            WSB = 4   # wsum mini-block: spread slots so the PE never
                      # lumps 32 of them into one ACT period
            deferred_wsum = []
            pending_exp = []
            exp_delay = [0]

            # storage slot s' = parity*32 + q*16 + r  ->  token chunk
            # k = 4r + 2*parity + q  (from the xbar transpose layout)
            def kphys(sl):
                parity, q, r = sl // 32, (sl // 16) % 2, sl % 16
                return 4 * r + 2 * parity + q

            def emit_wsum_block():
                b, blk = deferred_wsum.pop(0)
                _, xn_sb, aexp_sb = samples[b]
                av = aexp_sb.rearrange("p a b c d -> p (a b c) d")
                for k in range(WSB):
                    sl = blk * WSB + k
                    nc.tensor.matmul(
                        misc[0:O, 0:XD],
                        av[:, sl, :],
                        xn_sb[:, kphys(sl), :],
                        start=(sl == 0),
                        stop=(sl == NC64 - 1),
                    )
                if blk == NC64 // WSB - 1:
                    zinv_sb = smalls.tile([O, 1], f32, tag="zinv")
                    nc.vector.reciprocal(out=zinv_sb, in_=misc[0:O, D : D + 1])
                    o_sb = smalls.tile([O, D], f32, tag="osb")
                    nc.vector.tensor_scalar_mul(o_sb, misc[0:O, 0:D], zinv_sb)
                    nc.sync.dma_start(out=out_d[b], in_=o_sb)

            # main software-pipelined loop: per iteration emit the NEXT
            # group's projections before the PREVIOUS group's score matmuls
            # so the PE keeps the tanh stream (ACT) fed.
            pending = None
            for b in range(BPC):
                for i in range(O):
                    for g in range(NG):
                        if b == 0 and i == 0 and g == 0:
                            start_sample(b)
                        if i == 1 and g == 0 and b + 1 < BPC:
                            start_sample(b + 1)
                        xt_sb = samples[b][0]
                        z_ps = zp.tile([D, 2, 512], f32)
                        for h in range(2):
                            t0 = g * 1024 + h * 512
                            nc.tensor.matmul(
                                z_ps[:, h, :],
                                wu_sb[:, i, :],
                                xt_sb[:, t0 : t0 + 512],
                                start=True,
                                stop=True,
                            )
                        if pending is not None:
                            emit_wedot(*pending)
                        if pending_exp:
                            exp_delay[0] += 1
                            if exp_delay[0] > 2:
                                exp_delay[0] = 0
                                emit_exp(pending_exp.pop(0))
                        if deferred_wsum:
                            emit_wsum_block()
                        t_sb = sp.tile([D, 1024], bf16, tag="tanh")
                        j = i * BPC + b
                        nc.scalar.activation(
                            out=t_sb,
                            in_=z_ps.rearrange("p a f -> p (a f)"),
                            func=Tanh,
                            bias=ch_sb[:, j : j + 1],
                        )
                        pending = (b, i, g, t_sb)
            emit_wedot(*pending)
            while pending_exp:
                emit_exp(pending_exp.pop(0))
            while deferred_wsum:
                emit_wsum_block()

    nc.compile()
    return nc


def _prep_core_inputs(x, Wu, bu, Wv, We, last_nodes):
    """Host-side input marshalling: dtype cast + layout (weights pre-halved
    for the tanh formulation). Returns per-core input maps."""
    x = np.ascontiguousarray(x, dtype=np.float32)
    ln = np.asarray(last_nodes).astype(np.int64)
    xb = x.reshape(B, T, D)
    xbf = xb.astype(BF)                                  # [B, T, D] bf16
    xt = np.ascontiguousarray(xbf.transpose(0, 2, 1))    # [B, D, T]
    # xn layout with ones column: xn[b, p, c*129 + j] = xb[b, c*128 + p, j],
    # ones at j = 128
    xn4 = np.ones((B, NC64, D, XD), BF)
    xn4[:, :, :, 0:D] = xbf.reshape(B, NC64, D, D)
    # -> [B, p, c, j]: index p is the token-within-chunk = axis 2
    xn = np.ascontiguousarray(
        xn4.transpose(0, 2, 1, 3).reshape(B, D, NC64 * XD)
    )
    # x_last gather, transposed: xlT[d, j], j = i*BPC + b_local
    xl = xb[np.arange(B)[:, None], ln + np.arange(O)[None, :] * N]   # [B,O,D]
    wuT = np.ascontiguousarray((Wu * 0.5).transpose(2, 0, 1).astype(BF))
    wvT = np.ascontiguousarray((Wv * 0.5).transpose(2, 0, 1).astype(BF))
    # one-hot We stationary tiles: weoh[e, i, r, col] = We[i,e]/2 if col==r
    weoh = np.zeros((D, O, 16, 16), np.float32)
    for r in range(16):
        weoh[:, :, r, r] = (We * 0.5).T
    weoh = np.ascontiguousarray(weoh.astype(BF))
    # identity blocks for the PE transposes (rows 0-15 and 32-47)
    id16 = np.zeros((48, 16), np.float32)
    id16[0:16, :] = np.eye(16)
    id16[32:48, :] = np.eye(16)
    id16 = np.ascontiguousarray(id16.astype(BF))
    # bias bu replicated per (i, b) col: buR[e, i*BPC+b] = bu[i, e]/2
    buR = np.ascontiguousarray(
        np.repeat((bu * 0.5).astype(np.float32).T[:, :, None], BPC, axis=2
                  ).reshape(D, O * BPC)
    )

    maps = []
    for core in range(NCORES):
        sl = slice(core * BPC, (core + 1) * BPC)
        xlc = xl[sl]                                     # [BPC, O, D]
        xlT = np.ascontiguousarray(
            xlc.transpose(2, 1, 0).reshape(D, O * BPC).astype(BF)
        )
        maps.append(
            {
                "xt": xt[sl],
                "xn": xn[sl],
                "wuT": wuT,
                "wvT": wvT,
                "weoh": weoh,
                "buR": buR,
                "xlT": xlT,
                "id16": id16,
            }
        )
    return maps


_CACHE = {}
TRACE = False


def kernel(**inputs):
    x = np.asarray(inputs["x"])
    Wu = np.asarray(inputs["Wu"], dtype=np.float32)
    bu = np.asarray(inputs["bu"], dtype=np.float32)
    Wv = np.asarray(inputs["Wv"], dtype=np.float32)
    We = np.asarray(inputs["We"], dtype=np.float32)
    last_nodes = np.asarray(inputs["last_nodes"])

    maps = _prep_core_inputs(x, Wu, bu, Wv, We, last_nodes)
    if "nc" not in _CACHE:
        _CACHE["nc"] = _build_program()
    nc = _CACHE["nc"]
    res = run_bass_kernel_spmd(nc, maps, list(range(NCORES)), trace=TRACE)
    _CACHE["last_res"] = res
    outs = [np.asarray(r["out"], dtype=np.float32) for r in res.results]
    return np.concatenate(outs, axis=0)  # [B, O, D]


if __name__ == "__main__":
    rng = np.random.default_rng(0)
    x = rng.standard_normal((B, O, N, D), dtype=np.float32)
    Wu = rng.standard_normal((O, D, D), dtype=np.float32) * 0.09
    bu = np.zeros((O, D), np.float32)
    Wv = rng.standard_normal((O, D, D), dtype=np.float32) * 0.09
    We = rng.standard_normal((O, D), dtype=np.float32) * 0.09
    ln = rng.integers(0, N, size=(B, O)).astype(np.int64)
    out = kernel(x=x, Wu=Wu, bu=bu, Wv=Wv, We=We, last_nodes=ln)
    print(out.shape, out.dtype)


# revision 3
# speedup vs baseline: 1.0053x; 1.0053x over previous
"""AttnReadout kernel v2 for Trainium2, 8 NeuronCores, data-parallel over batch.

Math (per batch b, head i):
  c[i,e]    = (bu[i,e] + sum_d Wv[i,e,d] * x[b, i, last_nodes[b,i], d]) / 2
  z[t,e]    = sum_d x[b,t,d] * Wu[i,e,d] / 2       (t over O*N = 8192 tokens)
  s[t,e]    = tanh(z[t,e] + c[i,e])                 (sigmoid via tanh identity)
  score[t]  = sum_e (We[i,e]/2) * s[t,e]            (+ const, dropped: softmax
                                                     is shift invariant)
  alpha     = exp(score) / Z
  out[b,i]  = sum_t alpha[t] * x[b,t,:]

Differences vs v1 (120us):
  - wedot: instead of loading each 128-col tanh tile as PE stationary
    (~107ns LDWEIGHTS per 128 tokens, ~55us total), use one-hot-We
    stationary tiles [128,16] (16-col loads) with the tanh tile as a
    256-col moving operand. Scores land chunk-on-partition in PSUM.
    Chunk pairs run concurrently on col groups (0,0)/(0,32).
  - Z: a 129th all-ones column in the xn layout makes the weighted-sum
    matmul also produce Z = sum(alpha); drops ACT accumulator reads.
  - exp bias dropped (shift invariance).
  - tanh reads PSUM in 2048-elem instructions over a 6-bank z ring.
  - scores [48,256] -> bf16 (DVE) -> DMA xbar transposes [16,128]->[128,16]
    -> exp on [128,128] -> alpha in wsum stationary layout.
"""

import numpy as np
import ml_dtypes

import concourse.bacc as bacc
import concourse.tile as tile
from concourse import mybir
from concourse.bass_utils import run_bass_kernel_spmd

BF = ml_dtypes.bfloat16
B, O, N, D = 32, 2, 4096, 128
NCORES = 8
BPC = B // NCORES          # samples per core
T = O * N                  # tokens per sample
NG = 8                     # tanh groups of 1024 per (b, i)
CHUNK = 256                # wedot chunk (one-hot col r = c//2, parity = c%2)
NCH = T // CHUNK           # 32 wedot chunks per (b, i)
NC64 = T // 128            # 64 wsum chunks of 128 tokens
XD = D + 1                 # xn row length: 128 features + ones column


def _build_program():
    nc = bacc.Bacc("TRN2", target_bir_lowering=False)
    dt = mybir.dt
    f32, bf16 = dt.float32, dt.bfloat16

    xt_d = nc.dram_tensor("xt", [BPC, D, T], bf16, kind="ExternalInput")
    xn_d = nc.dram_tensor("xn", [BPC, D, NC64 * XD], bf16, kind="ExternalInput")
    wu_d = nc.dram_tensor("wuT", [D, O, D], bf16, kind="ExternalInput")
    wv_d = nc.dram_tensor("wvT", [D, O, D], bf16, kind="ExternalInput")
    woh_d = nc.dram_tensor("weoh", [D, O, 16, 16], bf16, kind="ExternalInput")
    bu_d = nc.dram_tensor("buR", [D, O * BPC], f32, kind="ExternalInput")
    xl_d = nc.dram_tensor("xlT", [D, O * BPC], bf16, kind="ExternalInput")
    id_d = nc.dram_tensor("id16", [48, 16], bf16, kind="ExternalInput")
    out_d = nc.dram_tensor("out", [BPC, O, D], f32, kind="ExternalOutput")

    Tanh = mybir.ActivationFunctionType.Tanh
    Exp = mybir.ActivationFunctionType.Exp

    with tile.TileContext(nc) as tc:
        from contextlib import ExitStack

        with ExitStack() as ctx:
            singles = ctx.enter_context(tc.tile_pool(name="singles", bufs=1))
            xtp = ctx.enter_context(tc.tile_pool(name="xtp", bufs=3))
            xnp = ctx.enter_context(tc.tile_pool(name="xnp", bufs=3))
            sp = ctx.enter_context(tc.tile_pool(name="sp", bufs=4))
            zp = ctx.enter_context(tc.tile_pool(name="zp", bufs=2, space="PSUM"))
            psp = ctx.enter_context(tc.tile_pool(name="psp", bufs=1, space="PSUM"))
            smalls = ctx.enter_context(tc.tile_pool(name="smalls", bufs=2))
            alphp = ctx.enter_context(tc.tile_pool(name="alphp", bufs=3))

            # PE warm-up source: no DMA dependency, ready immediately
            warm_sb = singles.tile([D, D], bf16)
            nc.gpsimd.memset(warm_sb, 0.001)

            # first group's xt ahead of the weights so the first
            # projection can start early; ch-path weights ride the idle
            # Activation DMA queue instead of the serial Sync queue
            xt0_sb = xtp.tile([D, T], bf16, tag="xt")
            nc.sync.dma_start(out=xt0_sb[:, 0:512], in_=xt_d[0, :, 0:512])
            nc.sync.dma_start(out=xt0_sb[:, 512:1024], in_=xt_d[0, :, 512:1024])

            wu_sb = singles.tile([D, O, D], bf16)
            nc.sync.dma_start(out=wu_sb, in_=wu_d[:])
            wv_sb = singles.tile([D, O, D], bf16)
            nc.scalar.dma_start(out=wv_sb, in_=wv_d[:])
            woh_sb = singles.tile([D, O, 16, 16], bf16)
            nc.sync.dma_start(out=woh_sb, in_=woh_d[:])
            bu_sb = singles.tile([D, O * BPC], f32)
            nc.scalar.dma_start(out=bu_sb, in_=bu_d[:])
            xl_sb = singles.tile([D, O * BPC], bf16)
            nc.scalar.dma_start(out=xl_sb, in_=xl_d[:])
            id_sb = singles.tile([48, 16], bf16)
            nc.sync.dma_start(out=id_sb, in_=id_d[:])

            # PSUM: z double-buffer (2x2 banks via zp pool) + two score
            # banks (one per wedot parity / PE column group) + misc (1 bank)
            # + alpha transpose bank
            scorA = psp.tile([16, CHUNK], f32, tag="scoresA")
            scorB = psp.tile([48, CHUNK], f32, tag="scoresB")
            misc = psp.tile([D, 512], f32, tag="misc")
            alps = psp.tile([D, O, 2, 2, 16], bf16, tag="alps")

            # HAM warmup: dense dummy matmuls fill the otherwise-idle PE
            # during the initial DMA wait so the real stream starts at the
            # un-throttled 2.4 GHz clock (the activity window is ~3.4 us).
            for w in range(16):
                nc.tensor.matmul(
                    misc[:, 0:D],
                    warm_sb,
                    warm_sb,
                    start=(w == 0),
                    stop=(w == 15),
                )

            # per-(sample, head) tanh bias ch[e, j] = (xv + bu)/2, j = i*BPC+b
            # (wv and bu are uploaded pre-halved)
            c_ps = misc[:, 136 : 136 + O * BPC]
            for i in range(O):
                nc.tensor.matmul(
                    c_ps[:, i * BPC : (i + 1) * BPC],
                    wv_sb[:, i, :],
                    xl_sb[:, i * BPC : (i + 1) * BPC],
                    start=True,
                    stop=True,
                )
            ch_sb = singles.tile([D, O * BPC], f32)
            nc.vector.tensor_add(ch_sb, c_ps, bu_sb)

            samples = {}

            def start_sample(b):
                xt_sb = xt0_sb if b == 0 else xtp.tile([D, T], bf16, tag="xt")
                # first sample: fine-grained leading slices so the first
                # projections can start before the bulk of the load lands
                bounds = (
                    [1024, 2048, 3072, 4096, 5120, 6144, T]
                    if b == 0
                    else [q * (T // 4) for q in range(4)] + [T]
                )
                for lo, hi in zip(bounds[:-1], bounds[1:]):
                    nc.sync.dma_start(out=xt_sb[:, lo:hi], in_=xt_d[b, :, lo:hi])
                xn_sb = xnp.tile([D, NC64, XD], bf16, tag="xn")
                for g in range(2):
                    nc.sync.dma_start(
                        out=xn_sb[:, g * 32 : (g + 1) * 32, :],
                        in_=xn_d[b, :, g * 32 * XD : (g + 1) * 32 * XD].rearrange(
                            "p (c d) -> p c d", c=32
                        ),
                    )
                # alpha tile [128, (parity, q, r, i)]: chunk-major so
                # the wsum stationary [128, 2] slices are contiguous
                aexp_sb = alphp.tile([D, 2, 2, 16, O], bf16, tag="aexp")
                samples[b] = (xt_sb, xn_sb, aexp_sb)

            # wedot: chunk c of (b,i): stationary woh[:, i, r=c//2, :] one-hot
            # (We/2 at col r), moving tanh [128, 256], accumulated into row r
            # of its parity's scores bank. Even/odd chunks pair on PE column
            # groups 0/32 (concurrent execution); separate banks keep the
            # has_written accumulate chains independent.
            def emit_wedot(b, i, g, t_flat):
                for sub in range(4):
                    c = g * 4 + sub
                    r, parity = c // 2, c % 2
                    out_ap = scorA if parity == 0 else scorB[32:48, :]
                    nc.tensor.matmul(
                        out_ap,
                        woh_sb[:, i, r, :],
                        t_flat[:, sub * CHUNK : (sub + 1) * CHUNK],
                        start=(c == parity),
                        stop=(c >= NCH - 2),
                    )
                if g == NG - 1:
                    emit_score_tail(b, i)

            # scores -> bf16 -> PE transposes (into the alps PSUM bank) ->
            # exp from PSUM into the SBUF alpha layout
            def emit_score_tail(b, i):
                aexp_sb = samples[b][2]
                scb = smalls.tile([48, CHUNK], bf16, tag="scb")
                nc.vector.tensor_copy(out=scb[0:16, :], in_=scorA)
                nc.vector.tensor_copy(out=scb[32:48, :], in_=scorB[32:48, :])
                for parity in range(2):
                    for q in range(2):
                        nc.tensor.transpose(
                            out=alps[:, i, parity, q, :],
                            in_=scb[parity * 32 : parity * 32 + 16,
                                    q * 128 : (q + 1) * 128],
                            identity=id_sb[parity * 32 : parity * 32 + 16, :],
                        )
                if i == O - 1:
                    pending_exp.append(b)

            def emit_exp(b):
                aexp_sb = samples[b][2]
                # out AP permutes head-major (transpose layout) into
                # chunk-major (contiguous wsum stationary slices)
                nc.scalar.activation(
                    out=aexp_sb.rearrange("p a b c d -> p d a b c"),
                    in_=alps,
                    func=Exp,
                )
                for blk in range(NC64 // WSB):
                    deferred_wsum.append((b, blk))

            WSB = 4   # wsum mini-block: spread slots so the PE never
                      # lumps 32 of them into one ACT period
            deferred_wsum = []
            pending_exp = []
            exp_delay = [0]

            # storage slot s' = parity*32 + q*16 + r  ->  token chunk
            # k = 4r + 2*parity + q  (from the xbar transpose layout)
            def kphys(sl):
                parity, q, r = sl // 32, (sl // 16) % 2, sl % 16
                return 4 * r + 2 * parity + q

            def emit_wsum_block():
                b, blk = deferred_wsum.pop(0)
                _, xn_sb, aexp_sb = samples[b]
                av = aexp_sb.rearrange("p a b c d -> p (a b c) d")
                for k in range(WSB):
                    sl = blk * WSB + k
                    nc.tensor.matmul(
                        misc[0:O, 0:XD],
                        av[:, sl, :],
                        xn_sb[:, kphys(sl), :],
                        start=(sl == 0),
                        stop=(sl == NC64 - 1),
                    )
                if blk == NC64 // WSB - 1:
                    zinv_sb = smalls.tile([O, 1], f32, tag="zinv")
                    nc.vector.reciprocal(out=zinv_sb, in_=misc[0:O, D : D + 1])
                    o_sb = smalls.tile([O, D], f32, tag="osb")
                    nc.vector.tensor_scalar_mul(o_sb, misc[0:O, 0:D], zinv_sb)
                    nc.sync.dma_start(out=out_d[b], in_=o_sb)

            # main software-pipelined loop: per iteration emit the NEXT
            # group's projections before the PREVIOUS group's score matmuls
            # so the PE keeps the tanh stream (ACT) fed.
            pending = None
            for b in range(BPC):
                for i in range(O):
                    for g in range(NG):
                        if b == 0 and i == 0 and g == 0:
                            start_sample(b)
                        if i == 1 and g == 0 and b + 1 < BPC:
                            start_sample(b + 1)
                        xt_sb = samples[b][0]
                        z_ps = zp.tile([D, 2, 512], f32)
                        for h in range(2):
                            t0 = g * 1024 + h * 512
                            nc.tensor.matmul(
                                z_ps[:, h, :],
                                wu_sb[:, i, :],
                                xt_sb[:, t0 : t0 + 512],
                                start=True,
                                stop=True,
                            )
                        if pending is not None:
                            emit_wedot(*pending)
                        if pending_exp:
                            exp_delay[0] += 1
                            if exp_delay[0] > 2:
                                exp_delay[0] = 0
                                emit_exp(pending_exp.pop(0))
                        if deferred_wsum:
                            emit_wsum_block()
                        t_sb = sp.tile([D, 1024], bf16, tag="tanh")
                        j = i * BPC + b
                        nc.scalar.activation(
                            out=t_sb,
                            in_=z_ps.rearrange("p a f -> p (a f)"),
                            func=Tanh,
                            bias=ch_sb[:, j : j + 1],
                        )
                        pending = (b, i, g, t_sb)
            emit_wedot(*pending)
            while pending_exp:
                emit_exp(pending_exp.pop(0))
            while deferred_wsum:
                emit_wsum_block()

    nc.compile()
    return nc


def _prep_core_inputs(x, Wu, bu, Wv, We, last_nodes):
    """Host-side input marshalling: dtype cast + layout (weights pre-halved
    for the tanh formulation). Returns per-core input maps."""
    x = np.ascontiguousarray(x, dtype=np.float32)
    ln = np.asarray(last_nodes).astype(np.int64)
    xb = x.reshape(B, T, D)
    xbf = xb.astype(BF)                                  # [B, T, D] bf16
    xt = np.ascontiguousarray(xbf.transpose(0, 2, 1))    # [B, D, T]
    # xn layout with ones column: xn[b, p, c*129 + j] = xb[b, c*128 + p, j],
    # ones at j = 128
    xn4 = np.ones((B, NC64, D, XD), BF)
    xn4[:, :, :, 0:D] = xbf.reshape(B, NC64, D, D)
    # -> [B, p, c, j]: index p is the token-within-chunk = axis 2
    xn = np.ascontiguousarray(
        xn4.transpose(0, 2, 1, 3).reshape(B, D, NC64 * XD)
    )
    # x_last gather, transposed: xlT[d, j], j = i*BPC + b_local
    xl = xb[np.arange(B)[:, None], ln + np.arange(O)[None, :] * N]   # [B,O,D]
    wuT = np.ascontiguousarray((Wu * 0.5).transpose(2, 0, 1).astype(BF))
    wvT = np.ascontiguousarray((Wv * 0.5).transpose(2, 0, 1).astype(BF))
    # one-hot We stationary tiles: weoh[e, i, r, col] = We[i,e]/2 if col==r
    weoh = np.zeros((D, O, 16, 16), np.float32)
    for r in range(16):
        weoh[:, :, r, r] = (We * 0.5).T
    weoh = np.ascontiguousarray(weoh.astype(BF))
    # identity blocks for the PE transposes (rows 0-15 and 32-47)
    id16 = np.zeros((48, 16), np.float32)
    id16[0:16, :] = np.eye(16)
    id16[32:48, :] = np.eye(16)
    id16 = np.ascontiguousarray(id16.astype(BF))
    # bias bu replicated per (i, b) col: buR[e, i*BPC+b] = bu[i, e]/2
    buR = np.ascontiguousarray(
        np.repeat((bu * 0.5).astype(np.float32).T[:, :, None], BPC, axis=2
                  ).reshape(D, O * BPC)
    )

    maps = []
    for core in range(NCORES):
        sl = slice(core * BPC, (core + 1) * BPC)
        xlc = xl[sl]                                     # [BPC, O, D]
        xlT = np.ascontiguousarray(
            xlc.transpose(2, 1, 0).reshape(D, O * BPC).astype(BF)
        )
        maps.append(
            {
                "xt": xt[sl],
                "xn": xn[sl],
                "wuT": wuT,
                "wvT": wvT,
                "weoh": weoh,
                "buR": buR,
                "xlT": xlT,
                "id16": id16,
            }
        )
    return maps


_CACHE = {}
TRACE = False


def kernel(**inputs):
    x = np.asarray(inputs["x"])
    Wu = np.asarray(inputs["Wu"], dtype=np.float32)
    bu = np.asarray(inputs["bu"], dtype=np.float32)
    Wv = np.asarray(inputs["Wv"], dtype=np.float32)
    We = np.asarray(inputs["We"], dtype=np.float32)
    last_nodes = np.asarray(inputs["last_nodes"])

    maps = _prep_core_inputs(x, Wu, bu, Wv, We, last_nodes)
    if "nc" not in _CACHE:
        _CACHE["nc"] = _build_program()
    nc = _CACHE["nc"]
    res = run_bass_kernel_spmd(nc, maps, list(range(NCORES)), trace=TRACE)
    _CACHE["last_res"] = res
    outs = [np.asarray(r["out"], dtype=np.float32) for r in res.results]
    return np.concatenate(outs, axis=0)  # [B, O, D]


if __name__ == "__main__":
    rng = np.random.default_rng(0)
    x = rng.standard_normal((B, O, N, D), dtype=np.float32)
    Wu = rng.standard_normal((O, D, D), dtype=np.float32) * 0.09
    bu = np.zeros((O, D), np.float32)
    Wv = rng.standard_normal((O, D, D), dtype=np.float32) * 0.09
    We = rng.standard_normal((O, D), dtype=np.float32) * 0.09
    ln = rng.integers(0, N, size=(B, O)).astype(np.int64)
    out = kernel(x=x, Wu=Wu, bu=bu, Wv=Wv, We=We, last_nodes=ln)
    print(out.shape, out.dtype)
